# revision 1
# baseline (speedup 1.0000x reference)
import numpy as np
import ml_dtypes

BF = ml_dtypes.bfloat16
B, N, WT, F, H = 64, 512, 24, 16, 128
NL = N // 8  # nodes per core


def _build(nc, bass, mybir, tile):
    f32 = mybir.dt.float32
    bf16 = mybir.dt.bfloat16
    Alu = mybir.AluOpType
    Act = mybir.ActivationFunctionType

    t_a1t = nc.dram_tensor("a1t", [128, 4, NL], bf16, kind="ExternalInput").ap()
    t_a2t = nc.dram_tensor("a2t", [128, 4, NL], bf16, kind="ExternalInput").ap()
    t_xm = nc.dram_tensor("xm", [B, 128, 4, WT * F], bf16, kind="ExternalInput").ap()
    t_xc0 = nc.dram_tensor("xc0", [128, 3, NL, B], bf16, kind="ExternalInput").ap()
    t_dc = nc.dram_tensor("dc", [128, 8, 3, 128], bf16, kind="ExternalInput").ap()
    t_wih = nc.dram_tensor("wih", [128, NL, 3, 128], bf16, kind="ExternalInput").ap()
    t_whh = nc.dram_tensor("whh", [128, NL, 3, 128], bf16, kind="ExternalInput").ap()
    t_brz = nc.dram_tensor("brz", [128, NL, 2], f32, kind="ExternalInput").ap()
    t_bin = nc.dram_tensor("bin", [128, NL], f32, kind="ExternalInput").ap()
    t_bhn = nc.dram_tensor("bhn", [128, NL], f32, kind="ExternalInput").ap()
    t_wout = nc.dram_tensor("wout", [128, F], bf16, kind="ExternalInput").ap()
    t_bout = nc.dram_tensor("bout", [128, F], f32, kind="ExternalInput").ap()
    t_out = nc.dram_tensor("out", [128, 32, F], f32, kind="ExternalOutput").ap()
    spill = nc.dram_tensor("spill", [WT, 128, NL, B], bf16).ap()

    with tile.TileContext(nc) as tc:
        with (
            tc.tile_pool(name="const", bufs=1) as cpool,
            tc.tile_pool(name="hpool", bufs=1) as hpool,
        ):
            a1t = cpool.tile([128, 4, NL], bf16)
            a2t = cpool.tile([128, 4, NL], bf16)
            dc = cpool.tile([128, 8, 3, 128], bf16)
            wih = cpool.tile([128, NL, 3, 128], bf16)
            whh = cpool.tile([128, NL, 3, 128], bf16)
            brz = cpool.tile([128, NL, 2], f32)
            bin_ = cpool.tile([128, NL], f32)
            bhn = cpool.tile([128, NL], f32)
            wout = cpool.tile([128, F], bf16)
            bout = cpool.tile([128, F], f32)
            for sb, dr in [(a1t, t_a1t), (a2t, t_a2t), (dc, t_dc), (wih, t_wih),
                           (whh, t_whh), (brz, t_brz), (bin_, t_bin), (bhn, t_bhn),
                           (wout, t_wout), (bout, t_bout)]:
                nc.sync.dma_start(sb[:], dr[:])
            h = hpool.tile([128, NL, B], bf16)
            nc.any.memset(h[:], 0.0)

            # ---- phase 1: diffusion hops + projection, spill sp to DRAM ----
            with tc.tile_pool(name="xc", bufs=1) as xcpool:
                xc0 = xcpool.tile([128, 3, NL, B], bf16)
                xc1 = xcpool.tile([128, 3, NL, B], bf16)
                xc2 = xcpool.tile([128, 3, NL, B], bf16)
                nc.sync.dma_start(xc0[:], t_xc0[:])
                with (
                    tc.tile_pool(name="xmb", bufs=2) as xmpool,
                    tc.tile_pool(name="p1", bufs=2, space="PSUM") as p1pool,
                ):
                    for b in range(B):
                        xmb = xmpool.tile([128, 4, WT * F], bf16)
                        nc.sync.dma_start(xmb[:], t_xm[b])
                        P1 = p1pool.tile([128, 3, NL], f32, tag="P1")
                        P2 = p1pool.tile([128, 3, NL], f32, tag="P2")
                        for mc in range(4):
                            for cc in range(3):
                                lhsT = xmb[:, mc, 128 * cc:128 * cc + 128]
                                nc.tensor.matmul(P1[:, cc, :], lhsT, a1t[:, mc, :],
                                                 start=(mc == 0), stop=(mc == 3))
                                nc.tensor.matmul(P2[:, cc, :], lhsT, a2t[:, mc, :],
                                                 start=(mc == 0), stop=(mc == 3))
                        nc.vector.tensor_copy(xc1[:, :, :, b], P1[:])
                        nc.scalar.copy(xc2[:, :, :, b], P2[:])

                with (
                    tc.tile_pool(name="pp", bufs=2, space="PSUM") as pppool,
                    tc.tile_pool(name="ev", bufs=3) as evpool,
                ):
                    xcs = [xc0, xc1, xc2]
                    for w in range(WT):
                        cc = w // 8
                        for half in range(2):
                            Pp = pppool.tile([128, 32, B], f32)
                            n0 = 32 * half
                            for hop in range(3):
                                rhs = xcs[hop][:, cc, n0:n0 + 32, :]
                                for q in range(4):
                                    nc.tensor.matmul(
                                        Pp[:, 8 * q:8 * q + 8, :],
                                        dc[:, w % 8, hop, :],
                                        rhs[:, 8 * q:8 * q + 8, :],
                                        start=(hop == 0), stop=(hop == 2))
                            ev = evpool.tile([128, 32, B], bf16)
                            nc.vector.tensor_copy(ev[:], Pp[:])
                            nc.sync.dma_start(spill[w, :, n0:n0 + 32, :], ev[:])

            # ---- phase 2: GRU over time ----
            with (
                tc.tile_pool(name="sp", bufs=2) as sppool,
                tc.tile_pool(name="g", bufs=2) as gpool,
                tc.tile_pool(name="gp", bufs=2, space="PSUM") as gppool,
            ):
                for w in range(WT):
                    spw = sppool.tile([128, NL, B], bf16)
                    nc.sync.dma_start(spw[:], spill[w])
                    for g in range(8):
                        P = gppool.tile([128, 8, 256], f32)
                        for j in range(8):
                            nn = 8 * g + j
                            sp_n = spw[:, nn, :]
                            h_n = h[:, nn, :]
                            for gc in range(2):
                                o = P[:, j, 64 * gc:64 * gc + 64]
                                nc.tensor.matmul(o, wih[:, nn, gc, :], sp_n,
                                                 start=True, stop=False)
                                nc.tensor.matmul(o, whh[:, nn, gc, :], h_n,
                                                 start=False, stop=True)
                            nc.tensor.matmul(P[:, j, 128:192], wih[:, nn, 2, :],
                                             sp_n, start=True, stop=True)
                            nc.tensor.matmul(P[:, j, 192:256], whh[:, nn, 2, :],
                                             h_n, start=True, stop=True)
                        sl = slice(8 * g, 8 * g + 8)
                        przt = gpool.tile([128, 8, 2, B], bf16, tag="prz")
                        prz_in = P[:, :, 0:128].rearrange("p j (t b) -> p j t b", t=2)
                        nc.vector.tensor_tensor(
                            przt[:], prz_in,
                            brz[:, sl, :, None].to_broadcast((128, 8, 2, B)), Alu.add)
                        rz = gpool.tile([128, 8, 2, B], bf16, tag="rz")
                        nc.scalar.activation(rz[:], przt[:], Act.Sigmoid)
                        hn1 = gpool.tile([128, 8, B], bf16, tag="hn1")
                        nc.vector.tensor_tensor(
                            hn1[:], P[:, :, 192:256],
                            bhn[:, sl, None].to_broadcast((128, 8, B)), Alu.add)
                        tt = gpool.tile([128, 8, B], bf16, tag="tt")
                        nc.vector.tensor_tensor(tt[:], rz[:, :, 0, :], hn1[:], Alu.mult)
                        ut = gpool.tile([128, 8, B], bf16, tag="ut")
                        nc.vector.tensor_tensor(ut[:], tt[:], P[:, :, 128:192], Alu.add)
                        qt = gpool.tile([128, 8, B], bf16, tag="qt")
                        nc.vector.tensor_tensor(
                            qt[:], ut[:],
                            bin_[:, sl, None].to_broadcast((128, 8, B)), Alu.add)
                        nt = gpool.tile([128, 8, B], bf16, tag="nt")
                        nc.scalar.activation(nt[:], qt[:], Act.Tanh)
                        st = gpool.tile([128, 8, B], bf16, tag="st")
                        nc.gpsimd.tensor_tensor(st[:], h[:, sl, :], nt[:], Alu.subtract)
                        vt = gpool.tile([128, 8, B], bf16, tag="vt")
                        nc.gpsimd.tensor_tensor(vt[:], rz[:, :, 1, :], st[:], Alu.mult)
                        nc.vector.tensor_tensor(h[:, sl, :], nt[:], vt[:], Alu.add)

            # ---- output projection ----
            with (
                tc.tile_pool(name="po", bufs=1, space="PSUM") as popool,
                tc.tile_pool(name="ou", bufs=1) as oupool,
            ):
                Po = popool.tile([128, 32, F], f32)
                for c in range(32):
                    nc.tensor.matmul(Po[:, c, :], h[:, 2 * c:2 * c + 2, :], wout[:],
                                     start=True, stop=True)
                outsb = oupool.tile([128, 32, F], f32)
                nc.vector.tensor_tensor(
                    outsb[:], Po[:], bout[:, None, :].to_broadcast((128, 32, F)),
                    Alu.add)
                nc.sync.dma_start(t_out[:], outsb[:])
    nc.compile()


def kernel(**inputs):
    import concourse.bacc as bacc
    import concourse.bass as bass
    import concourse.mybir as mybir
    import concourse.tile as tile
    from concourse import bass_utils

    x = np.asarray(inputs["x"], np.float32)
    A = np.asarray(inputs["A_fw"], np.float32)
    dcw = np.asarray(inputs["dc_weights"], np.float32)
    W_ih = np.asarray(inputs["W_ih"], np.float32)
    W_hh = np.asarray(inputs["W_hh"], np.float32)
    b_ih = np.asarray(inputs["b_ih"], np.float32)
    b_hh = np.asarray(inputs["b_hh"], np.float32)
    W_out = np.asarray(inputs["W_out"], np.float32)
    b_out = np.asarray(inputs["b_out"], np.float32)

    A2 = A @ A
    dc_all = np.stack([dcw[0:16], dcw[16:32] + dcw[32:48], dcw[48:64] + dcw[64:80]])
    xbf = x.astype(BF)
    xm = np.ascontiguousarray(xbf.reshape(B, 4, 128, WT * F).transpose(0, 2, 1, 3))
    dcm = np.zeros((128, 8, 3, 128), np.float32)
    for wo in range(8):
        dcm[wo * 16:wo * 16 + 16, wo] = dc_all.transpose(1, 0, 2)
    dc_host = dcm.astype(BF)
    wout_h = W_out.astype(BF)
    bout_h = np.tile(b_out[None, :], (128, 1)).astype(np.float32)

    in_maps = []
    for c in range(8):
        ns = slice(c * NL, (c + 1) * NL)
        a1t = np.ascontiguousarray(
            A[ns].T.astype(BF).reshape(4, 128, NL).transpose(1, 0, 2))
        a2t = np.ascontiguousarray(
            A2[ns].T.astype(BF).reshape(4, 128, NL).transpose(1, 0, 2))
        xl = xbf[:, ns]  # [b, n, w, f]
        xc0 = np.ascontiguousarray(
            xl.reshape(B, NL, WT * F).transpose(2, 1, 0)
            .reshape(3, 128, NL, B).transpose(1, 0, 2, 3))
        wih_h = np.ascontiguousarray(
            W_ih[ns].transpose(2, 0, 1).astype(BF).reshape(128, NL, 3, 128))
        whh_h = np.ascontiguousarray(
            W_hh[ns].transpose(2, 0, 1).astype(BF).reshape(128, NL, 3, 128))
        br = (b_ih[ns, 0:128] + b_hh[ns, 0:128]).T
        bz = (b_ih[ns, 128:256] + b_hh[ns, 128:256]).T
        brz_h = np.ascontiguousarray(np.stack([br, bz], axis=-1)).astype(np.float32)
        bin_h = np.ascontiguousarray(b_ih[ns, 256:384].T).astype(np.float32)
        bhn_h = np.ascontiguousarray(b_hh[ns, 256:384].T).astype(np.float32)
        in_maps.append({
            "a1t": a1t, "a2t": a2t, "xm": xm, "xc0": xc0, "dc": dc_host,
            "wih": wih_h, "whh": whh_h, "brz": brz_h, "bin": bin_h, "bhn": bhn_h,
            "wout": wout_h, "bout": bout_h,
        })

    nc = bacc.Bacc("TRN2", target_bir_lowering=False, debug=False, num_devices=8)
    _build(nc, bass, mybir, tile)
    res = bass_utils.run_bass_kernel_spmd(nc, in_maps, core_ids=list(range(8)))
    import os, time
    if os.environ.get("DGCN_BENCH"):
        for it in range(int(os.environ["DGCN_BENCH"])):
            t0 = time.time()
            res = bass_utils.run_bass_kernel_spmd(nc, in_maps, core_ids=list(range(8)))
            print(f"bench iter {it}: {(time.time()-t0)*1e3:.1f} ms", flush=True)

    out = np.zeros((B, N, F), np.float32)
    for c in range(8):
        arr = res.results[c]["out"]  # [128, 32, F]
        tmp = arr.transpose(1, 0, 2).reshape(32, 2, B, F).transpose(2, 0, 1, 3)
        out[:, c * NL:(c + 1) * NL] = tmp.reshape(B, NL, F)
    return out



# revision 2
# speedup vs baseline: 2.5563x; 2.5563x over previous
import numpy as np
import ml_dtypes

BF = ml_dtypes.bfloat16
B, N, WT, F, H = 64, 512, 24, 16, 128
NL = N // 8  # nodes per core


def _build(nc, bass, mybir, tile):
    f32 = mybir.dt.float32
    bf16 = mybir.dt.bfloat16
    Alu = mybir.AluOpType
    Act = mybir.ActivationFunctionType

    t_a12 = nc.dram_tensor("a12", [128, 4, 2, NL], bf16, kind="ExternalInput").ap()
    t_xm = nc.dram_tensor("xm", [3, 128, B, 4, 128], bf16, kind="ExternalInput").ap()
    t_xq0 = nc.dram_tensor("xq0", [WT, 17, NL, B], bf16, kind="ExternalInput").ap()
    t_vih = nc.dram_tensor("vih", [49, NL, 4, 128], bf16, kind="ExternalInput").ap()
    t_whh = nc.dram_tensor("whh", [128, NL, 3, 128], bf16, kind="ExternalInput").ap()
    t_wout = nc.dram_tensor("wout", [128, F], bf16, kind="ExternalInput").ap()
    t_bout = nc.dram_tensor("bout", [128, F], f32, kind="ExternalInput").ap()
    t_out = nc.dram_tensor("out", [128, 32, F], f32, kind="ExternalOutput").ap()
    spill1 = nc.dram_tensor("spill1", [3, 128, NL, B], bf16).ap()
    spill2 = nc.dram_tensor("spill2", [3, 128, NL, B], bf16).ap()

    with tile.TileContext(nc) as tc:
        with (
            tc.tile_pool(name="const", bufs=1) as cpool,
            tc.tile_pool(name="hpool", bufs=1) as hpool,
        ):
            vih = cpool.tile([49, NL, 4, 128], bf16)
            whh = cpool.tile([128, NL, 3, 128], bf16)
            wout = cpool.tile([128, F], bf16)
            bout = cpool.tile([128, F], f32)
            a12 = cpool.tile([128, 4, 2, NL], bf16)
            nc.sync.dma_start(a12[:], t_a12[:])
            hb0 = hpool.tile([128, NL, B], bf16)
            hb1 = hpool.tile([128, NL, B], bf16)
            hbuf = [hb0, hb1]
            nc.any.memset(hb0[:], 0.0)

            with (
                tc.tile_pool(name="xs", bufs=1) as xspool,
                tc.tile_pool(name="xmb", bufs=4) as xmpool,
                tc.tile_pool(name="xq", bufs=3) as xqpool,
                tc.tile_pool(name="g", bufs=3) as gpool,
                tc.tile_pool(name="gp", bufs=2, space="PSUM") as gppool,
            ):
                # phase 1 (per cc chunk): hop matmuls -> SBUF staging ->
                # DRAM spill; interleaved ahead of the GRU chunk loop
                def hops_cc(cc):
                    x1 = xspool.tile([128, NL, B], bf16, tag="xc1")
                    x2 = xspool.tile([128, NL, B], bf16, tag="xc2")
                    for b4 in range(B // 4):
                        xmb = xmpool.tile([128, 4, 4, 128], bf16)
                        nc.sync.dma_start(xmb[:], t_xm[cc, :, 4 * b4:4 * b4 + 4])
                        P12 = gppool.tile([128, 8, 2, B], f32, tag="pni")
                        for bi in range(4):
                            for mc in range(4):
                                nc.tensor.matmul(
                                    P12[:, bi, :, :], xmb[:, bi, mc, :],
                                    a12[:, mc, :, :],
                                    start=(mc == 0), stop=(mc == 3))
                        bs = slice(4 * b4, 4 * b4 + 4)
                        pr1 = P12[:, 0:4, 0, :].rearrange("p b n -> p n b")
                        pr2 = P12[:, 0:4, 1, :].rearrange("p b n -> p n b")
                        nc.vector.tensor_copy(x1[:, :, bs], pr1)
                        nc.scalar.copy(x2[:, :, bs], pr2)
                    nc.sync.dma_start(spill1[cc], x1[:])
                    nc.sync.dma_start(spill2[cc], x2[:])

                hops_cc(0)
                for sb, dr in [(vih, t_vih), (whh, t_whh), (wout, t_wout),
                               (bout, t_bout)]:
                    nc.sync.dma_start(sb[:], dr[:])

                # phase 2: GRU over time; projection + biases folded into vih
                for w in range(WT):
                    wo, cc = w % 8, w // 8
                    if w == 1:
                        hops_cc(1)
                    if w == 9:
                        hops_cc(2)
                    h = hbuf[w % 2]
                    hnew = hbuf[1 - w % 2]
                    xq = xqpool.tile([49, NL, B], bf16)
                    nc.gpsimd.dma_start(xq[0:17], t_xq0[w])
                    nc.gpsimd.dma_start(xq[17:33],
                                        spill1[cc, 16 * wo:16 * wo + 16])
                    nc.gpsimd.dma_start(xq[33:49],
                                        spill2[cc, 16 * wo:16 * wo + 16])
                    for g in range(8):
                        P_rz = gppool.tile([128, 8, 2, B], f32, tag="prz")
                        # P_ni: per node j: [0]=hn  [1]=i_n
                        P_ni = gppool.tile([128, 8, 2, B], f32, tag="pni")
                        for j in range(8):
                            nn = 8 * g + j
                            xq_n = xq[:, nn, :]
                            h_n = h[:, nn, :]
                            for gc in range(2):
                                o = P_rz[:, j, gc, :]
                                nc.tensor.matmul(o, vih[:, nn, gc, :], xq_n,
                                                 start=True, stop=False)
                                nc.tensor.matmul(o, whh[:, nn, gc, :], h_n,
                                                 start=False, stop=True)
                        for j in range(8):
                            nn = 8 * g + j
                            nc.tensor.matmul(P_ni[:, j, 0, :], vih[:, nn, 3, :],
                                             xq[:, nn, :], start=True, stop=False)
                            nc.tensor.matmul(P_ni[:, j, 0, :], whh[:, nn, 2, :],
                                             h[:, nn, :], start=False, stop=True)
                        for j in range(8):
                            nn = 8 * g + j
                            nc.tensor.matmul(P_ni[:, j, 1, :], vih[:, nn, 2, :],
                                             xq[:, nn, :], start=True, stop=True)
                        sl = slice(8 * g, 8 * g + 8)
                        # r = sigmoid(r-in); zbar = sigmoid(-z-in) = 1-z
                        rz = gpool.tile([128, 8, 2, B], bf16, tag="rz")
                        nc.scalar.activation(rz[:], P_rz[:], Act.Sigmoid)
                        tt = gpool.tile([128, 8, B], bf16, tag="tt")
                        nc.vector.tensor_tensor(tt[:], rz[:, :, 0, :],
                                                P_ni[:, :, 0, :], Alu.mult)
                        nin = gpool.tile([128, 8, B], bf16, tag="nin")
                        nc.vector.tensor_tensor(nin[:], tt[:], P_ni[:, :, 1, :],
                                                Alu.add)
                        nt = gpool.tile([128, 8, B], bf16, tag="nt")
                        nc.scalar.activation(nt[:], nin[:], Act.Tanh)
                        dt_ = gpool.tile([128, 8, B], bf16, tag="dt")
                        nc.vector.tensor_tensor(dt_[:], nt[:], h[:, sl, :],
                                                Alu.subtract)
                        et = gpool.tile([128, 8, B], bf16, tag="et")
                        nc.vector.tensor_tensor(et[:], rz[:, :, 1, :], dt_[:],
                                                Alu.mult)
                        nc.gpsimd.tensor_tensor(hnew[:, sl, :], h[:, sl, :],
                                                et[:], Alu.add)

            # ---- output projection ----
            with (
                tc.tile_pool(name="po", bufs=1, space="PSUM") as popool,
                tc.tile_pool(name="ou", bufs=1) as oupool,
            ):
                hT = hbuf[WT % 2]
                Po = popool.tile([128, 32, F], f32)
                for c in range(32):
                    nc.tensor.matmul(Po[:, c, :], hT[:, 2 * c:2 * c + 2, :],
                                     wout[:], start=True, stop=True)
                outsb = oupool.tile([128, 32, F], f32)
                nc.vector.tensor_tensor(
                    outsb[:], Po[:], bout[:, None, :].to_broadcast((128, 32, F)),
                    Alu.add)
                nc.sync.dma_start(t_out[:], outsb[:])
    nc.compile()


def kernel(**inputs):
    import concourse.bacc as bacc
    import concourse.bass as bass
    import concourse.mybir as mybir
    import concourse.tile as tile
    from concourse import bass_utils

    x = np.asarray(inputs["x"], np.float32)
    A = np.asarray(inputs["A_fw"], np.float32)
    dcw = np.asarray(inputs["dc_weights"], np.float32)
    W_ih = np.asarray(inputs["W_ih"], np.float32)
    W_hh = np.asarray(inputs["W_hh"], np.float32)
    b_ih = np.asarray(inputs["b_ih"], np.float32)
    b_hh = np.asarray(inputs["b_hh"], np.float32)
    W_out = np.asarray(inputs["W_out"], np.float32)
    b_out = np.asarray(inputs["b_out"], np.float32)

    A2 = A @ A
    # fw/bw hops share A, so pair weights collapse: [3, F, H]
    dc_all = np.stack([dcw[0:16], dcw[16:32] + dcw[32:48], dcw[48:64] + dcw[64:80]])
    xbf = x.astype(BF)
    # xm[cc, p, b, mc, f] with p = node-in-chunk(mc), (cc,f) = wf split
    xm = np.ascontiguousarray(
        xbf.reshape(B, 4, 128, 3, 128).transpose(3, 2, 0, 1, 4))
    wout_h = W_out.astype(BF)
    bout_h = np.tile(b_out[None, :], (128, 1)).astype(np.float32)

    # V[n] = dc folded into per-node input weights: [N, 3hops*F, 3H]
    V = np.einsum("ofh,ngh->nofg", dc_all, W_ih).reshape(N, 48, 384)

    in_maps = []
    for c in range(8):
        ns = slice(c * NL, (c + 1) * NL)
        a1t = A[ns].T.astype(BF).reshape(4, 128, NL).transpose(1, 0, 2)
        a2t = A2[ns].T.astype(BF).reshape(4, 128, NL).transpose(1, 0, 2)
        a12 = np.ascontiguousarray(np.stack([a1t, a2t], axis=2))
        # xq0: [W, 17, NL, B]; rows 0..15 = local x (hop0), row 16 = ones
        xl = x[:, ns]  # [B, NL, W, F]
        xq0 = np.empty((WT, 17, NL, B), np.float32)
        xq0[:, 0:16] = xl.transpose(2, 3, 1, 0)
        xq0[:, 16] = 1.0
        # vih: [49, NL, 4, 128]; partition rows: 0..15 hop0-f, 16 bias,
        # 17..32 hop1-f, 33..48 hop2-f. chunks: r, z(negated), n, hb(bhn)
        vih_h = np.zeros((49, NL, 4, 128), np.float32)
        Vl = V[ns]  # [NL, 48, 384]
        for c4 in range(3):
            gs = slice(128 * c4, 128 * c4 + 128)
            vih_h[0:16, :, c4, :] = Vl[:, 0:16, gs].transpose(1, 0, 2)
            vih_h[17:33, :, c4, :] = Vl[:, 16:32, gs].transpose(1, 0, 2)
            vih_h[33:49, :, c4, :] = Vl[:, 32:48, gs].transpose(1, 0, 2)
        bsum = b_ih[ns] + b_hh[ns]
        vih_h[16, :, 0, :] = bsum[:, 0:128]
        vih_h[16, :, 1, :] = bsum[:, 128:256]
        vih_h[16, :, 2, :] = b_ih[ns, 256:384]
        vih_h[16, :, 3, :] = b_hh[ns, 256:384]
        # negate z gate so sigmoid yields (1 - z) directly
        vih_h[:, :, 1, :] *= -1.0
        whh_l = W_hh[ns].transpose(2, 0, 1).reshape(128, NL, 3, 128).copy()
        whh_l[:, :, 1, :] *= -1.0
        whh_h = np.ascontiguousarray(whh_l.astype(BF))
        in_maps.append({
            "a12": a12, "xm": xm, "xq0": xq0.astype(BF),
            "vih": vih_h.astype(BF), "whh": whh_h,
            "wout": wout_h, "bout": bout_h,
        })

    nc = bacc.Bacc("TRN2", target_bir_lowering=False, debug=False, num_devices=8)
    _build(nc, bass, mybir, tile)
    res = bass_utils.run_bass_kernel_spmd(nc, in_maps, core_ids=list(range(8)))

    out = np.zeros((B, N, F), np.float32)
    for c in range(8):
        arr = res.results[c]["out"]  # [128, 32, F]
        tmp = arr.transpose(1, 0, 2).reshape(32, 2, B, F).transpose(2, 0, 1, 3)
        out[:, c * NL:(c + 1) * NL] = tmp.reshape(B, NL, F)
    return out
